# revision 30
# baseline (speedup 1.0000x reference)
"""MoE-LoRA linear kernel for TRN2, data-parallel over tokens across 8 cores.

Per-core computation (Tc tokens, D=1024, E=10, R=4, TOP_K=2):
  base = x @ W^T + b ; logits = x @ gateW^T + gb ; top2 softmax -> dense w[t,e]
  h = (x @ lora_down^T) * w  (rank-expanded) ; out = base + 0.25 * h @ lora_up^T

Layout: x is transposed on the HOST (so no on-chip PE transposes of x) and all
matmul operands are bf16 (1 cyc/row on the PE, half the HBM traffic; measured
end-to-end rel err ~6.5e-3 against the fp32 reference, tolerance 2e-2).
The gate is merged into the lora-down lhsT (M=74 columns), so gating logits
cost zero extra PE time. Top-2 softmax runs on the DVE in the shadow of the
first token-tile's base matmuls.
"""

import contextlib
import ctypes
import sys
import types

import numpy as np

SO_PATH = "/opt/axon/libaxon_pjrt.so"

D = 1024
E = 10
R = 4
ER = E * R          # 40
GO = 64             # gate rows start (PSUM partition reads must be 32-aligned)
GC = GO + E         # 74 lhsT columns: down 0..39, zeros 40..63, gate 64..73
TT = 4              # 128-token tiles per group
TG = 128 * TT       # 512 tokens per group


def install_ntff_hook():
    """run_bass_kernel_spmd(trace=True) needs antenv.axon_hooks; synthesize it."""
    if "antenv.axon_hooks" in sys.modules:
        return
    def _ntff_profile_via_ctypes(so_path):
        lib = ctypes.CDLL(so_path)
        if not hasattr(lib, "axon_start_nrt_profile"):
            return None
        lib.axon_start_nrt_profile.argtypes = [ctypes.POINTER(ctypes.c_int64), ctypes.c_size_t]
        lib.axon_start_nrt_profile.restype = ctypes.c_int64
        lib.axon_stop_nrt_profile.argtypes = [ctypes.c_char_p]
        lib.axon_stop_nrt_profile.restype = ctypes.c_int64

        @contextlib.contextmanager
        def _hook(output_dir, device_ids):
            import jax
            jax.devices()
            if device_ids:
                ids = (ctypes.c_int64 * len(device_ids))(*device_ids)
                rc = lib.axon_start_nrt_profile(ids, len(device_ids))
            else:
                rc = lib.axon_start_nrt_profile(None, 0)
            if rc != 0:
                raise RuntimeError(f"axon_start_nrt_profile rc={rc}")
            try:
                yield
            finally:
                n = lib.axon_stop_nrt_profile(str(output_dir).encode())
                if n < 0:
                    raise RuntimeError(f"axon_stop_nrt_profile rc={n}")
        return _hook

    mod = types.ModuleType("antenv.axon_hooks")
    mod.get_axon_ntff_profile_hook = lambda: _ntff_profile_via_ctypes(SO_PATH)
    sys.modules["antenv.axon_hooks"] = mod


def build_kernel(Tc, n_cores=8):
    import concourse.bass as bass  # noqa: F401
    import concourse.mybir as mybir
    import concourse.tile as tile
    from concourse import bacc
    from concourse.bass import ds, ts
    from concourse.masks import make_identity

    f32 = mybir.dt.float32
    bf16 = mybir.dt.bfloat16
    NG = Tc // TG
    assert Tc % TG == 0

    nc = bacc.Bacc("TRN2", target_bir_lowering=False, debug=False, num_devices=n_cores)

    # x^T (host-transposed, bf16): [kb, p, t] with d = kb*128 + p. Strided
    # per-partition lines fan each transfer across all 16 DMA rings.
    xt_in = nc.declare_dram_parameter("xt", [8, 128, Tc], bf16, isOutput=False)
    wt_in = nc.declare_dram_parameter("wt", [128, 8, D], bf16, isOutput=False)
    g_in = nc.declare_dram_parameter("g", [128, 8, GC], bf16, isOutput=False)
    u_in = nc.declare_dram_parameter("u", [ER + 1, D], bf16, isOutput=False)
    gb_in = nc.declare_dram_parameter("gb", [E, 1], f32, isOutput=False)
    out_dram = nc.declare_dram_parameter("out", [Tc, D], bf16, isOutput=True)

    with tile.TileContext(nc) as tc:
        with contextlib.ExitStack() as ctx:
            singles = ctx.enter_context(tc.tile_pool(name="singles", bufs=1))
            xtp = ctx.enter_context(tc.tile_pool(name="xtp", bufs=2))
            smallp = ctx.enter_context(tc.tile_pool(name="smallp", bufs=2))
            h1p = ctx.enter_context(tc.tile_pool(name="h1p", bufs=2))
            outp = ctx.enter_context(tc.tile_pool(name="outp", bufs=4))
            pgp = ctx.enter_context(tc.tile_pool(name="pgp", bufs=1, space="PSUM"))
            pslp = ctx.enter_context(tc.tile_pool(name="pslp", bufs=1, space="PSUM"))
            pswp = ctx.enter_context(tc.tile_pool(name="pswp", bufs=1, space="PSUM"))
            po = ctx.enter_context(tc.tile_pool(name="po", bufs=5, space="PSUM"))

            # ---- constants ----
            g_sb = singles.tile([128, 8, GC], bf16)
            wt_sb = singles.tile([128, 8, D], bf16)
            u_sb = singles.tile([ER + 1, D], bf16)
            gb_sb = singles.tile([E, 1], f32)
            ident = singles.tile([128, 128], f32)
            ones = singles.tile([ER + 1, 1], bf16)

            nc.sync.dma_start(out=g_sb[:], in_=g_in[:])
            make_identity(nc, ident)
            nc.vector.memset(ones[:], 1.0)

            def load_x(gi):
                xg = xtp.tile([128, 8, TG], bf16, tag="xg")
                for kb in range(8):
                    nc.sync.dma_start(out=xg[:, kb, :], in_=xt_in[kb, :, ds(gi * TG, TG)])
                return xg

            # group-0 x slabs queued before the 2MB wt: the DMA rings serve
            # transfers roughly in issue order, and the first (down+gate)
            # matmuls need only x slabs + g
            xg_next = load_x(0)
            for kb in range(8):
                nc.sync.dma_start(out=wt_sb[:, kb, :], in_=wt_in[:, kb, :])
            nc.sync.dma_start(out=gb_sb[:], in_=gb_in[:])

            for gi in range(NG):
                xg = xg_next
                # ---- lora-down + gate logits in one accumulation (M=74) ----
                pg = pgp.tile([GC, TG], f32, tag="pg")
                for kb in range(8):
                    nc.tensor.matmul(
                        pg[:], g_sb[:, kb, :], xg[:, kb, :],
                        start=(kb == 0), stop=(kb == 7),
                    )
                if gi + 1 < NG:
                    xg_next = load_x(gi + 1)
                if gi == 0:
                    # u isn't needed until the first up matmul (~25us in);
                    # keep it out of the head's DMA stream
                    nc.sync.dma_start(out=u_sb[:], in_=u_in[:])
                hr = smallp.tile([ER, TG], f32, tag="hr")
                nc.scalar.copy(hr[:], pg[0:ER, :])
                lt3 = smallp.tile([E, TG], f32, tag="lt3")
                nc.vector.tensor_scalar_add(lt3[:], pg[GO:GC, :], gb_sb[:])

                # logits -> [token, e] for the top-2 softmax (DVE ops run
                # with all 128 lanes active in this layout)
                psl = pslp.tile([128, TT, E], f32, tag="psl")
                for tt in range(TT):
                    nc.tensor.transpose(
                        psl[:, tt, :], lt3[:, ts(tt, 128)], ident[0:E, 0:E],
                    )

                # ---- top-2 softmax on DVE (hidden under tt=0 base matmuls) ----
                L = smallp.tile([128, TT, E], f32, tag="L")
                nc.vector.tensor_copy(L[:], psl[:])
                m1 = smallp.tile([128, TT], f32, tag="m1")
                nc.vector.reduce_max(m1[:], L[:], axis=mybir.AxisListType.X)
                Lm = smallp.tile([128, TT, E], f32, tag="Lm")
                nc.vector.tensor_tensor(
                    Lm[:], L[:], m1[:, :, None].to_broadcast(L.shape),
                    mybir.AluOpType.subtract,
                )
                mmax = smallp.tile([128, TT, E], f32, tag="mmax")
                nc.vector.tensor_scalar(
                    mmax[:], Lm[:], 0.0, None, op0=mybir.AluOpType.is_equal,
                )
                nc.vector.tensor_scalar_mul(mmax[:], mmax[:], -1e30)
                nc.vector.tensor_tensor(mmax[:], Lm[:], mmax[:], mybir.AluOpType.add)
                m2 = smallp.tile([128, TT], f32, tag="m2")
                nc.vector.reduce_max(m2[:], mmax[:], axis=mybir.AxisListType.X)
                mask2 = smallp.tile([128, TT, E], f32, tag="mask2")
                nc.vector.tensor_tensor(
                    mask2[:], Lm[:], m2[:, :, None].to_broadcast(Lm.shape),
                    mybir.AluOpType.is_ge,
                )
                ex = smallp.tile([128, TT, E], f32, tag="ex")
                nc.scalar.activation(ex[:], Lm[:], mybir.ActivationFunctionType.Exp)
                nc.vector.tensor_tensor(ex[:], ex[:], mask2[:], mybir.AluOpType.mult)
                zsum = smallp.tile([128, TT], f32, tag="zsum")
                nc.vector.reduce_sum(zsum[:], ex[:], axis=mybir.AxisListType.X)
                nc.vector.reciprocal(zsum[:], zsum[:])
                wfull = smallp.tile([128, TT, E], f32, tag="wfull")
                nc.vector.tensor_tensor(
                    wfull[:], ex[:], zsum[:, :, None].to_broadcast(ex.shape),
                    mybir.AluOpType.mult,
                )
                # expand over rank: [128, tt, e] -> [128, tt, e, r] (er, e-major)
                w40 = smallp.tile([128, TT, ER], f32, tag="w40")
                nc.vector.tensor_copy(
                    w40[:], wfull[:, :, :, None].to_broadcast([128, TT, E, R]),
                )

                def base_tt(tt):
                    p0 = po.tile([128, 512], f32, tag="po")
                    p1 = po.tile([128, 512], f32, tag="po")
                    for kb in range(8):
                        nc.tensor.matmul(
                            p0[:], xg[:, kb, ts(tt, 128)], wt_sb[:, kb, ds(0, 512)],
                            start=(kb == 0), stop=False,
                        )
                        nc.tensor.matmul(
                            p1[:], xg[:, kb, ts(tt, 128)], wt_sb[:, kb, ds(512, 512)],
                            start=(kb == 0), stop=False,
                        )
                    return p0, p1

                def up_pair(tt_pairs, h1):
                    # all up matmuls of the pair back-to-back (one PE run),
                    # then per-chunk copy + DMA so output drains early
                    for tt, (p0, p1) in tt_pairs:
                        for ch, p in enumerate((p0, p1)):
                            nc.tensor.matmul(
                                p[:], h1[:, ts(tt, 128)], u_sb[:, ds(ch * 512, 512)],
                                start=False, stop=True,
                            )
                    for tt, (p0, p1) in tt_pairs:
                        for ch, p in enumerate((p0, p1)):
                            o_c = outp.tile([128, 512], bf16, tag="o_c")
                            # split copies across scalar+vector so the final
                            # group's output drain isn't serialized on one
                            if ch == 0:
                                nc.scalar.copy(o_c[:], p[:])
                            else:
                                nc.vector.tensor_copy(o_c[:], p[:])
                            nc.sync.dma_start(
                                out=out_dram[
                                    ds(gi * TG + tt * 128, 128), ds(ch * 512, 512)
                                ],
                                in_=o_c[:],
                            )

                # tt=0/1 base matmuls run while the DVE computes the softmax;
                # the w-transpose + batched up matmuls land on the PE after
                pouts0 = base_tt(0)
                pouts1 = base_tt(1)
                psw = pswp.tile([ER, TG], f32, tag="psw")
                for tt in range(TT):
                    nc.tensor.transpose(psw[:, ts(tt, 128)], w40[:, tt, :], ident)
                h1 = h1p.tile([ER + 1, TG], bf16, tag="h1")
                nc.vector.tensor_copy(h1[:], ones.to_broadcast([ER + 1, TG]))
                nc.vector.tensor_tensor(
                    h1[0:ER, :], hr[:], psw[:], mybir.AluOpType.mult,
                )
                # tt2's base matmuls emitted before the first up pair: gives
                # the PE ready work to hide the psw->h1 (DVE) latency
                pouts2 = base_tt(2)
                up_pair([(0, pouts0), (1, pouts1)], h1)
                pouts3 = base_tt(3)
                up_pair([(2, pouts2), (3, pouts3)], h1)

    nc.compile()
    return nc


def pack_weights(W_base, b_base, gate_W, gate_b, lora_down, lora_up):
    """Host-side packing of the replicated weights into device layouts."""
    import ml_dtypes
    bf = ml_dtypes.bfloat16
    W_base = np.asarray(W_base, np.float32)
    b_base = np.asarray(b_base, np.float32)
    gate_W = np.asarray(gate_W, np.float32)
    gate_b = np.asarray(gate_b, np.float32)
    lora_down = np.asarray(lora_down, np.float32)
    lora_up = np.asarray(lora_up, np.float32)

    # wt[p, kb, o] = W^T[d, o] = W_base[o, d], d = kb*128+p
    wt = np.ascontiguousarray(
        np.ascontiguousarray(W_base.T).reshape(8, 128, D).transpose(1, 0, 2)
    ).astype(bf)
    # merged lhsT: cols 0..39 lora_down^T (e-major over rank), 40..49 gate_W^T
    G = np.zeros((D, GC), np.float32)
    G[:, 0:ER] = lora_down.reshape(ER, D).T
    G[:, GO:GC] = gate_W.T
    g = np.ascontiguousarray(G.reshape(8, 128, GC).transpose(1, 0, 2)).astype(bf)
    # u rows 0..39: lora_up[e, o, r]*0.25 -> [er, o]; row 40: b_base
    U = lora_up.transpose(0, 2, 1).reshape(ER, D) * (1.0 / R)
    u = np.ascontiguousarray(np.concatenate([U, b_base[None, :]], axis=0)).astype(bf)
    gb = np.ascontiguousarray(gate_b[:, None])
    return {"wt": wt, "g": g, "u": u, "gb": gb}


def run(nc, inputs, Tc, n_cores=8, trace=False):
    """Shard x over cores (host-transposed, bf16), run SPMD, gather output."""
    import ml_dtypes
    from concourse.bass_utils import run_bass_kernel_spmd

    bf = ml_dtypes.bfloat16
    x = np.asarray(inputs["x"], np.float32)
    B, S, _ = x.shape
    xf = x.reshape(B * S, D).astype(bf)
    assert B * S == Tc * n_cores
    packed = pack_weights(
        inputs["W_base"], inputs["b_base"], inputs["gate_W"],
        inputs["gate_b"], inputs["lora_down"], inputs["lora_up"],
    )
    in_maps = [
        {
            "xt": np.ascontiguousarray(xf[c * Tc:(c + 1) * Tc].T).reshape(8, 128, Tc),
            **packed,
        }
        for c in range(n_cores)
    ]
    kwargs = {}
    if trace:
        install_ntff_hook()
        kwargs = {"trace": True}
    res = run_bass_kernel_spmd(nc, in_maps, core_ids=list(range(n_cores)), **kwargs)
    out = np.concatenate(
        [np.asarray(res.results[c]["out"], np.float32) for c in range(n_cores)], axis=0
    )
    return out.reshape(B, S, D), res


_NC_CACHE = {}


def kernel(**inputs):
    """Full-input MoE-LoRA forward on 8 TRN2 NeuronCores (token-parallel).

    Takes the unsharded inputs from setup_inputs(), returns [B, S, D] fp32.
    """
    x = np.asarray(inputs["x"], np.float32)
    B, S, _ = x.shape
    n_cores = 8
    total = B * S
    assert total % n_cores == 0
    Tc = total // n_cores
    key = (Tc, n_cores)
    if key not in _NC_CACHE:
        _NC_CACHE[key] = build_kernel(Tc, n_cores=n_cores)
    nc = _NC_CACHE[key]
    last_err = None
    for _ in range(3):  # transient device wedges recover on retry
        try:
            out, _res = run(nc, inputs, Tc, n_cores=n_cores)
            return out
        except Exception as e:  # noqa: BLE001
            last_err = e
            import time as _time
            _time.sleep(5)
    raise last_err
